# revision 30
# baseline (speedup 1.0000x reference)
"""Trainium2 Bass kernel for 16-head causal self-attention with RoPE.

Problem: x:[2,2048,2048] -> MHA(wq,wk,wv,wo, causal mask, RoPE) -> [2,2048,2048].

Sharding (8 NeuronCores): core = b*4 + g, where b in {0,1} is the batch
(data parallel) and g in {0..3} is a head group of 4 heads (tensor parallel
over the 16 heads / 2048 channels: group g owns channels [g*512, (g+1)*512)).

Design (HW ~363us vs ~509us f32r baseline, same harness; sim ~314us):
  - bf16 on every DMA/SBUF path (fp32 PSUM accumulation): halves HBM bytes,
    removes the f32r narrow-matmul 4x penalty, enables DVE 2x/4x modes.
  - q/k/v stay SBUF-resident between projection and attention (no DRAM
    roundtrip): saves 24MB/core of DMA traffic.
  - coalesced DMAs (~34 per iteration vs ~350): HWDGE charges ~650ns of
    serialized descriptor-gen per DMA instruction.
  - full x resident in SBUF; projections run d-outer so one Ldweights
    (stationary load) serves the 4 s-chunk matmuls, and dedup_ldweights()
    strips the redundant reloads post-compile (PE keeps the stationary) —
    on HW each Ldweights+Matmult pair costs well above the 1-cycle/row
    model, so instruction count matters as much as MACs.
  - RoPE runs entirely on DVE: rotate-half = stream_shuffle partition-pair
    swap with the sign folded into sin host-side (no rotation matmul, no
    PSUM round-trip), applied on whole [128,2048] rows.
  - output stores go on the ACT queue and xf/wq SBUF tiles persist across
    the iteration so next iteration's loads are neither queued behind this
    iteration's stores nor blocked by SBUF-address reuse (those two effects
    serialized iterations by ~30us each).
  - phase C PSUM->SBUF copies split DVE/ACT so neither engine serializes
    the output stream.
Per-core phases:
  A: q/k = RoPE(x @ wq_loc.T), v = x @ wv_loc.T   (bf16 matmuls, SBUF out)
  B: per head, scoresT = kT.T-tiles @ qT, exp on ACT (scale folded),
     causal strictly-upper 128x512 tiles skipped, diagonal masked by
     in-place multiply with one shared 128x128 triangle, PV + ones-matmul
     denominator accumulate in PSUM, one reciprocal+mul per (head, chunk).
  C: partial out = ao @ wo_loc.T -> DRAM (bf16)
Host: out[b] = sum of the 4 group partials + bo.
Matmul moving dim is capped at 512 (one PSUM bank) by the ISA — wider
PSUM accumulators are written as multiple <=512-wide matmuls.
"""

import math
import sys

sys.path.insert(0, "/opt/trn_rl_repo")

import numpy as np

N_CORES = 8
B, S, D = 2, 2048, 2048
H, DH = 16, 128
G = 4                 # head groups (tensor-parallel factor per batch)
HPG = H // G          # heads per group = 4
CW = HPG * DH         # channels per group = 512
NT = S // 128         # 16 s-tiles
SC = 512              # free-dim chunk (one PSUM bank of fp32)
NQ = S // SC          # 4 s-chunks

# stream_shuffle mask: swap partition pairs (2i <-> 2i+1) per quadrant
SWAP_MASK = [i + 1 if i % 2 == 0 else i - 1 for i in range(32)]

_NC_CACHE: dict = {}


def dedup_ldweights(nc) -> int:
    """Remove InstLdweights that reload the stationary already in the PE.

    The PE array keeps the stationary across MultiplyMoving passes, so a
    Ldweights whose weights AP matches the previous one on the same block
    is a no-op that still costs ~128 PE rows. Only drop instructions with
    no semaphore waits/updates; anything else on the PE queue (other than
    Matmult / event-semaphore markers) conservatively resets tracking.
    """
    from concourse import mybir

    removed = 0
    for fn in nc.m.functions:
        for blk in fn.blocks:
            insts = blk.instructions
            last_key = None
            keep = []
            i = 0
            while i < len(insts):
                inst = insts[i]
                nm = type(inst).__name__
                if nm == "InstLdweights":
                    si = inst.sync_info
                    has_update = si is not None and len(si.on_update) > 0
                    waits = list(si.on_wait) if si is not None else []
                    key = (str(inst.ins[0]), str(inst.perf_mode),
                           str(inst.is_transpose),
                           str(getattr(inst, "tile_position", None)),
                           str(getattr(inst, "tile_size", None)))
                    nxt = insts[i + 1] if i + 1 < len(insts) else None
                    if (key == last_key and not has_update
                            and (not waits
                                 or type(nxt).__name__ == "InstMatmult")):
                        if waits:
                            # push the waits down onto the paired Matmult
                            nsi = nxt.sync_info
                            if nsi is None:
                                nxt.sync_info = mybir.SyncInfo(
                                    on_wait=waits, on_update=[])
                            else:
                                nsi.on_wait = waits + list(nsi.on_wait)
                        removed += 1
                        i += 1
                        continue
                    last_key = key
                elif nm not in ("InstMatmult", "InstEventSemaphore"):
                    if getattr(inst, "engine", None) is not None and \
                            str(inst.engine) == "EngineType.PE":
                        last_key = None
                keep.append(inst)
                i += 1
            blk.instructions[:] = keep
    return removed


def build_attn_nc(iters: int = 1, phases: int = 3):
    """Build + compile the Bass module (same program for all 8 cores)."""
    import concourse.tile as tile
    from concourse import bacc, mybir

    f32 = mybir.dt.float32
    bf16 = mybir.dt.bfloat16
    AF = mybir.ActivationFunctionType
    SCALE = 1.0 / math.sqrt(DH)

    nc = bacc.Bacc("TRN2", target_bir_lowering=False, debug=False,
                   num_devices=N_CORES)

    xT = nc.dram_tensor("xT", [D, S], bf16, kind="ExternalInput").ap()
    wqT = nc.dram_tensor("wqT", [D, CW], bf16, kind="ExternalInput").ap()
    wkT = nc.dram_tensor("wkT", [D, CW], bf16, kind="ExternalInput").ap()
    wvT = nc.dram_tensor("wvT", [D, CW], bf16, kind="ExternalInput").ap()
    woT = nc.dram_tensor("woT", [CW, D], bf16, kind="ExternalInput").ap()
    bqr = nc.dram_tensor("bqr", [HPG, DH, 1], f32, kind="ExternalInput").ap()
    bkr = nc.dram_tensor("bkr", [HPG, DH, 1], f32, kind="ExternalInput").ap()
    bvb = nc.dram_tensor("bvb", [128, CW], f32, kind="ExternalInput").ap()
    cosT = nc.dram_tensor("cosT", [DH, S], bf16, kind="ExternalInput").ap()
    sinT = nc.dram_tensor("sinT", [DH, S], bf16, kind="ExternalInput").ap()
    onesd = nc.dram_tensor("onesd", [128, 128], bf16, kind="ExternalInput").ap()
    mskT = nc.dram_tensor("mskT", [4, 128, SC], bf16, kind="ExternalInput").ap()

    out = nc.dram_tensor("out", [S, D], bf16, kind="ExternalOutput").ap()

    # rearranged DRAM views for coalesced loads
    xTr = xT.rearrange("(n p) s -> p n s", p=128)          # [128, 16, 2048]
    wqr = wqT.rearrange("(n p) c -> p n c", p=128)         # [128, 16, 512]
    wkr = wkT.rearrange("(n p) c -> p n c", p=128)
    wvr = wvT.rearrange("(n p) c -> p n c", p=128)
    wor = woT.rearrange("(h p) d -> p h d", p=128)         # [128, 4, 2048]

    with tile.TileContext(nc) as tc:
        SC2 = 2 * SC
        for it in range(iters):
            with tc.tile_pool(name="const", bufs=1) as cpool, \
                 tc.tile_pool(name="qkv", bufs=1) as qkvp, \
                 tc.tile_pool(name="xqpool", bufs=1) as xqpool:
                # ---- persistent-by-tag SBUF residents ----
                # DMA issue order puts wq + x-chunk-0 first so the first
                # projection matmuls start as early as possible
                w_sb = {}
                wq_t = cpool.tile([128, NT * CW], bf16, name=f"wq_{it}",
                                  tag="wq")
                nc.sync.dma_start(wq_t[:], wqr)
                w_sb["q"] = wq_t

                # q/k rope'd chunks and v tiles, SBUF-resident through phase B
                qh = {}   # (nm, ct, C) -> [DH, SC2] bf16
                vh = {}   # (qi, st) -> [128, CW] bf16

                # ---------------- phase A: projections + RoPE ----------
                # q/k accumulate in 1024-wide PSUM tiles (2 banks each):
                # halves the matmul/Ldweights count vs 512-wide
                with tc.tile_pool(name="wkvp", bufs=1) as wkvp, \
                     tc.tile_pool(name="prawp", bufs=2) as prawp, \
                     tc.tile_pool(name="workA", bufs=1) as wkp, \
                     tc.tile_pool(name="psA", bufs=1, space="PSUM") as psA, \
                     tc.tile_pool(name="psV", bufs=4, space="PSUM") as psV:
                    # full x resident in SBUF: enables d-outer loops where
                    # one weight-slice Ldweights serves all 4 s-chunk
                    # matmuls (stationary reuse -> dedup_ldweights)
                    xf = xqpool.tile([128, NT * S], bf16,
                                     name=f"xf_{it}", tag="xf")
                    nc.sync.dma_start(xf[:], xTr)
                    for nm, dram in (("k", wkr), ("v", wvr)):
                        t = wkvp.tile([128, NT * CW], bf16,
                                      name=f"w{nm}_{it}", tag=f"w{nm}")
                        nc.sync.dma_start(t[:], dram)
                        w_sb[nm] = t
                    cos_sb = cpool.tile([DH, S], bf16, name=f"cos_{it}",
                                        tag="cos")
                    nc.sync.dma_start(cos_sb[:], cosT[:])
                    sin_sb = cpool.tile([DH, S], bf16, name=f"sin_{it}",
                                        tag="sin")
                    nc.sync.dma_start(sin_sb[:], sinT[:])
                    bq_sb, bk_sb = [], []
                    for ct in range(HPG):
                        tq = cpool.tile([DH, 1], f32, name=f"bq{ct}_{it}",
                                        tag=f"bq{ct}")
                        nc.sync.dma_start(tq[:], bqr[ct])
                        bq_sb.append(tq)
                        tk = cpool.tile([DH, 1], f32, name=f"bk{ct}_{it}",
                                        tag=f"bk{ct}")
                        nc.sync.dma_start(tk[:], bkr[ct])
                        bk_sb.append(tk)
                    bvb_sb = cpool.tile([128, CW], f32, name=f"bvb{it}",
                                        tag="bvb")
                    nc.sync.dma_start(bvb_sb[:], bvb[:])
                    ones_sb = cpool.tile([128, 128], bf16, name=f"ones{it}",
                                         tag="ones")
                    nc.sync.dma_start(ones_sb[:], onesd[:])
                    # one [128,128] triangular diag-block mask
                    # (rr-independent)
                    tri_sb = cpool.tile([128, 128], bf16, name=f"tri_{it}",
                                        tag="tri")
                    nc.sync.dma_start(tri_sb[:], mskT[0, :, 0:128])

                    def qk_ct(nm, ct, bias_t):
                        # d-outer: one stationary w-slice per d feeds the 4
                        # s-chunk matmuls; all 4 chunks accumulate in one
                        # 4-bank PSUM tile (each matmul stays <=512 wide)
                        ps = psA.tile([128, NQ * SC], f32,
                                      name=f"ps{nm}{ct}_{it}", tag="ps")
                        for d in range(NT):
                            wsl = w_sb[nm][:, d * CW + ct * DH:
                                           d * CW + (ct + 1) * DH]
                            for qi in range(NQ):
                                nc.tensor.matmul(
                                    ps[:, qi * SC:(qi + 1) * SC], wsl,
                                    xf[:, d * S + qi * SC:
                                       d * S + (qi + 1) * SC],
                                    start=(d == 0), stop=(d == NT - 1),
                                    skip_group_check=True)
                        praw = prawp.tile(
                            [128, NQ * SC], bf16,
                            name=f"praw{nm}{ct}_{it}", tag="praw")
                        nc.vector.tensor_scalar_add(
                            praw[:], ps[:], bias_t[:])
                        return praw

                    def rope_ct(nm, ct, praw):
                        # RoPE on DVE only, whole-row [128, 2048] ops:
                        # rotate-half is a pairwise partition swap
                        # (stream_shuffle, quadrant-local) with the sign
                        # folded into sinT host-side
                        pro = qkvp.tile([DH, NQ * SC], bf16,
                                        name=f"pro{nm}{ct}_{it}",
                                        tag=f"pro{nm}{ct}")
                        sh = wkp.tile([128, NQ * SC], bf16,
                                      name=f"sh{nm}{ct}_{it}", tag="sh")
                        nc.vector.stream_shuffle(sh[:], praw[:], SWAP_MASK)
                        m1 = wkp.tile([128, NQ * SC], bf16,
                                      name=f"m1{nm}{ct}_{it}", tag="m1")
                        nc.vector.tensor_mul(m1[:], praw[:], cos_sb[:])
                        m2 = wkp.tile([128, NQ * SC], bf16,
                                      name=f"m2{nm}{ct}_{it}", tag="m2")
                        nc.vector.tensor_mul(m2[:], sh[:], sin_sb[:])
                        nc.vector.tensor_add(pro[:], m1[:], m2[:])
                        for qi in range(NQ):
                            qh[(nm, ct, qi)] = (pro, qi * SC)

                    def v_pair(sp):
                        psa = psV.tile([128, CW], f32,
                                       name=f"psv{sp}_{it}", tag="psv")
                        psb = psV.tile([128, CW], f32,
                                       name=f"psv{sp+1}_{it}", tag="psv")
                        for d in range(NT):
                            xc = d * S
                            nc.tensor.matmul(
                                psa[:],
                                xf[:, xc + sp * 128:xc + (sp + 1) * 128],
                                w_sb["v"][:, d * CW:(d + 1) * CW],
                                start=(d == 0), stop=(d == NT - 1))
                            nc.tensor.matmul(
                                psb[:],
                                xf[:, xc + (sp + 1) * 128:
                                   xc + (sp + 2) * 128],
                                w_sb["v"][:, d * CW:(d + 1) * CW],
                                start=(d == 0), stop=(d == NT - 1))
                        for st, ps in ((sp, psa), (sp + 1, psb)):
                            vt = qkvp.tile([128, CW], bf16,
                                           name=f"vt{st}_{it}",
                                           tag=f"vt{st}")
                            nc.vector.tensor_add(vt[:], ps[:], bvb_sb[:])
                            vh[(st // 4, st % 4)] = vt

                    # interleave v pair-groups (own PSUM banks) between the
                    # 8 q/k ct-groups so each group's PSUM drain is hidden
                    # behind the next group's matmuls
                    sp = 0
                    for nm, biases in (("q", bq_sb), ("k", bk_sb)):
                        for ct in range(HPG):
                            praw = qk_ct(nm, ct, biases[ct])
                            v_pair(sp)
                            sp += 2
                            rope_ct(nm, ct, praw)

                # ---------------- phase B: attention -------------------
                if phases < 2:
                    # debug mode: phase A only
                    continue
                with tc.tile_pool(name="aopool", bufs=1) as aopool, \
                     tc.tile_pool(name="wop", bufs=1) as wop:
                    aoT = aopool.tile([128, HPG * S], bf16, name=f"aoT_{it}",
                                      tag="aoT")
                    wo_sb = wop.tile([128, HPG * D], bf16, name=f"wo_{it}",
                                     tag="wo")
                    nc.sync.dma_start(wo_sb[:], wor)
                    with tc.tile_pool(name="atpool", bufs=8) as atpool, \
                         tc.tile_pool(name="recpool", bufs=2) as recpool, \
                         tc.tile_pool(name="psO", bufs=2, space="PSUM") as psO, \
                         tc.tile_pool(name="psS", bufs=4, space="PSUM") as psS:
                        for h in range(HPG):
                            qh_c = [qh[("q", h, qi)] for qi in range(NQ)]
                            kh_c = [qh[("k", h, qi)] for qi in range(NQ)]
                            vh_t = [vh[(t_ // 4, t_ % 4)]
                                    for t_ in range(NT)]
                            for c in range(NQ):
                                q0 = c * SC
                                ntile = 4 * c + 4
                                oT = psO.tile([DH, SC], f32,
                                              name=f"oT{h}{c}_{it}", tag="oT")
                                dn = psO.tile([128, SC], f32,
                                              name=f"dn{h}{c}_{it}", tag="dn")
                                n_den = 2 * c + 4   # paired fulls + 4 diag
                                den_i = 0
                                at_prev = None
                                for t_ in range(ntile):
                                    rr = t_ - 4 * c
                                    n0 = rr * 128 if rr > 0 else 0
                                    sps = psS.tile(
                                        [128, SC], f32,
                                        name=f"sps{h}{c}{t_}_{it}", tag="sps")
                                    kt, ko = kh_c[t_ // 4]
                                    qt, qo = qh_c[c]
                                    nc.tensor.matmul(
                                        sps[:, n0:],
                                        kt[:, ko + (t_ % 4) * 128:
                                           ko + (t_ % 4 + 1) * 128],
                                        qt[:, qo + n0:qo + SC],
                                        start=True, stop=True)
                                    at = atpool.tile(
                                        [128, SC], bf16,
                                        name=f"at{h}{c}{t_}_{it}", tag="at")
                                    nc.scalar.activation(
                                        at[:, n0:], sps[:, n0:],
                                        AF.Exp, bias=0.0, scale=SCALE)
                                    if rr >= 0:
                                        # mask the triangular diagonal block
                                        # in place (cols [0,n0) already
                                        # skipped entirely); the 128x128
                                        # triangle is rr-independent
                                        nc.vector.tensor_mul(
                                            at[:, n0:n0 + 128],
                                            at[:, n0:n0 + 128],
                                            tri_sb[:])
                                    nc.tensor.matmul(
                                        oT[:, n0:],
                                        vh_t[t_][:, h * DH:(h + 1) * DH],
                                        at[:, n0:],
                                        start=(t_ == 0),
                                        stop=(t_ == ntile - 1),
                                        skip_group_check=True)
                                    # denominator: pre-sum full-tile pairs
                                    # elementwise (den sums over all k, so
                                    # at_a+at_b first is exact modulo one
                                    # bf16 rounding) -> one ones-matmul per
                                    # pair instead of per tile
                                    if rr < 0 and t_ % 2 == 0:
                                        at_prev = at
                                        continue
                                    if rr < 0:
                                        ats = atpool.tile(
                                            [128, SC], bf16,
                                            name=f"ats{h}{c}{t_}_{it}",
                                            tag="ats")
                                        nc.vector.tensor_add(
                                            ats[:], at_prev[:], at[:])
                                        den_rhs = ats[:]
                                    else:
                                        den_rhs = at[:, n0:]
                                    nc.tensor.matmul(
                                        dn[:, n0:], ones_sb[:], den_rhs,
                                        start=(den_i == 0),
                                        stop=(den_i == n_den - 1),
                                        skip_group_check=True)
                                    den_i += 1
                                rec = recpool.tile([128, SC], f32,
                                                   name=f"rec{h}{c}_{it}",
                                                   tag="rec")
                                nc.vector.reciprocal(rec[:], dn[:])
                                nc.vector.tensor_mul(
                                    aoT[:, h * S + q0:h * S + q0 + SC],
                                    oT[:], rec[:])

                    # ------------ phase C: output projection ------------
                    if phases < 3:
                        continue
                    with tc.tile_pool(name="outpool", bufs=2) as outpool, \
                         tc.tile_pool(name="psC", bufs=8, space="PSUM") as psC:
                        for st in range(NT):
                            ops = []
                            for dc in range(4):
                                op = psC.tile([128, SC], f32,
                                              name=f"op{st}{dc}_{it}",
                                              tag="op")
                                ops.append(op)
                            for hh in range(HPG):
                                lhs = aoT[:, hh * S + st * 128:
                                          hh * S + (st + 1) * 128]
                                for dc in range(4):
                                    nc.tensor.matmul(
                                        ops[dc][:], lhs,
                                        wo_sb[:, hh * D + dc * SC:
                                              hh * D + (dc + 1) * SC],
                                        start=(hh == 0), stop=(hh == HPG - 1))
                            ot = outpool.tile([128, D], bf16,
                                              name=f"ot{st}_{it}", tag="ot")
                            for dc in range(4):
                                # split PSUM->SBUF copies across DVE and ACT
                                dst = ot[:, dc * SC:(dc + 1) * SC]
                                if dc % 2 == 0:
                                    nc.vector.tensor_copy(dst, ops[dc][:])
                                else:
                                    nc.scalar.copy(dst, ops[dc][:])
                            # store on the ACT queue (idle in phase C) so
                            # next iteration's loads on SP aren't queued
                            # behind these stores (cross-iter PE stall)
                            nc.scalar.dma_start(
                                out[st * 128:(st + 1) * 128, :], ot[:])
    nc.compile()
    dedup_ldweights(nc)
    return nc


def host_prep(inputs: dict) -> list:
    """Build per-core input maps (host-side sharding + relayout)."""
    import ml_dtypes
    bf16 = ml_dtypes.bfloat16

    x = np.asarray(inputs["x"], dtype=np.float32)
    wq = np.asarray(inputs["wq"], dtype=np.float32)
    wk = np.asarray(inputs["wk"], dtype=np.float32)
    wv = np.asarray(inputs["wv"], dtype=np.float32)
    wo = np.asarray(inputs["wo"], dtype=np.float32)
    bq = np.asarray(inputs["bq"], dtype=np.float32)
    bk = np.asarray(inputs["bk"], dtype=np.float32)
    bv = np.asarray(inputs["bv"], dtype=np.float32)
    mask = np.asarray(inputs["mask"])

    inv = 1.0 / (10000.0 ** (np.arange(0, DH, 2, dtype=np.float64) / DH))
    ang = np.arange(S, dtype=np.float64)[:, None] * inv[None, :]
    sin = np.repeat(np.sin(ang), 2, axis=1).astype(np.float32)
    cos = np.repeat(np.cos(ang), 2, axis=1).astype(np.float32)
    cosT = np.ascontiguousarray(cos.T).astype(bf16)
    # rotate-half is done as a pure pair-swap (stream_shuffle); fold the
    # sign of -x[2i+1] into sin: even dh rows get -sin
    sinS = np.ascontiguousarray(sin.T)
    sinS[0::2, :] *= -1.0
    sinT = sinS.astype(bf16)

    m2 = mask[0, 0]
    mskT = np.zeros((4, 128, SC), np.float32)
    for rr in range(4):
        # keep[i, j] = not masked(q=j, k=rr*128+i)
        mskT[rr] = (~m2[:SC, rr * 128:(rr + 1) * 128]).T.astype(np.float32)
    mskT = mskT.astype(bf16)

    xTb = [np.ascontiguousarray(x[b].T).astype(bf16) for b in range(B)]
    in_maps = []
    for core in range(N_CORES):
        b, g = divmod(core, G)
        c0 = g * CW
        in_maps.append({
            "xT": xTb[b],
            "wqT": np.ascontiguousarray(wq[c0:c0 + CW, :].T).astype(bf16),
            "wkT": np.ascontiguousarray(wk[c0:c0 + CW, :].T).astype(bf16),
            "wvT": np.ascontiguousarray(wv[c0:c0 + CW, :].T).astype(bf16),
            "woT": np.ascontiguousarray(wo[:, c0:c0 + CW].T).astype(bf16),
            "bqr": np.ascontiguousarray(
                bq[c0:c0 + CW].reshape(HPG, DH, 1)),
            "bkr": np.ascontiguousarray(
                bk[c0:c0 + CW].reshape(HPG, DH, 1)),
            "bvb": np.ascontiguousarray(
                np.broadcast_to(bv[c0:c0 + CW], (128, CW))),
            "cosT": cosT,
            "sinT": sinT,
            "onesd": np.ones((128, 128), np.float32).astype(bf16),
            "mskT": mskT,
        })
    return in_maps


def _get_nc():
    if "nc" not in _NC_CACHE:
        _NC_CACHE["nc"] = build_attn_nc(iters=1)
    return _NC_CACHE["nc"]


def kernel(**inputs) -> np.ndarray:
    from concourse.bass_utils import run_bass_kernel_spmd

    nc = _get_nc()
    in_maps = host_prep(inputs)
    res = run_bass_kernel_spmd(nc, in_maps, core_ids=list(range(N_CORES)))
    bo = np.asarray(inputs["bo"], dtype=np.float32)
    outp = np.zeros((B, S, D), np.float32)
    for core in range(N_CORES):
        outp[core // G] += np.asarray(res.results[core]["out"],
                                      dtype=np.float32)
    outp += bo[None, None, :]
    return outp


# revision 31
# speedup vs baseline: 1.1101x; 1.1101x over previous
"""Trainium2 Bass kernel for 16-head causal self-attention with RoPE.

Problem: x:[2,2048,2048] -> MHA(wq,wk,wv,wo, causal mask, RoPE) -> [2,2048,2048].

Sharding (8 NeuronCores): core = b*4 + g, where b in {0,1} is the batch
(data parallel) and g in {0..3} is a head group of 4 heads (tensor parallel
over the 16 heads / 2048 channels: group g owns channels [g*512, (g+1)*512)).

Design (HW ~363us vs ~509us f32r baseline, same harness; sim ~314us):
  - bf16 on every DMA/SBUF path (fp32 PSUM accumulation): halves HBM bytes,
    removes the f32r narrow-matmul 4x penalty, enables DVE 2x/4x modes.
  - q/k/v stay SBUF-resident between projection and attention (no DRAM
    roundtrip): saves 24MB/core of DMA traffic.
  - coalesced DMAs (~34 per iteration vs ~350): HWDGE charges ~650ns of
    serialized descriptor-gen per DMA instruction.
  - full x resident in SBUF; projections run d-outer so one Ldweights
    (stationary load) serves the 4 s-chunk matmuls, and dedup_ldweights()
    strips the redundant reloads post-compile (PE keeps the stationary) —
    on HW each Ldweights+Matmult pair costs well above the 1-cycle/row
    model, so instruction count matters as much as MACs.
  - RoPE runs entirely on DVE: rotate-half = stream_shuffle partition-pair
    swap with the sign folded into sin host-side (no rotation matmul, no
    PSUM round-trip), applied on whole [128,2048] rows.
  - output stores go on the ACT queue and xf/wq SBUF tiles persist across
    the iteration so next iteration's loads are neither queued behind this
    iteration's stores nor blocked by SBUF-address reuse (those two effects
    serialized iterations by ~30us each).
  - phase C PSUM->SBUF copies split DVE/ACT so neither engine serializes
    the output stream.
Per-core phases:
  A: q/k = RoPE(x @ wq_loc.T), v = x @ wv_loc.T   (bf16 matmuls, SBUF out)
  B: per head, scoresT = kT.T-tiles @ qT, exp on ACT (scale folded),
     causal strictly-upper 128x512 tiles skipped, diagonal masked by
     in-place multiply with one shared 128x128 triangle, PV + ones-matmul
     denominator accumulate in PSUM, one reciprocal+mul per (head, chunk).
  C: partial out = ao @ wo_loc.T -> DRAM (bf16)
Host: out[b] = sum of the 4 group partials + bo.
Matmul moving dim is capped at 512 (one PSUM bank) by the ISA — wider
PSUM accumulators are written as multiple <=512-wide matmuls.
"""

import math
import sys

sys.path.insert(0, "/opt/trn_rl_repo")

import numpy as np

N_CORES = 8
B, S, D = 2, 2048, 2048
H, DH = 16, 128
G = 4                 # head groups (tensor-parallel factor per batch)
HPG = H // G          # heads per group = 4
CW = HPG * DH         # channels per group = 512
NT = S // 128         # 16 s-tiles
SC = 512              # free-dim chunk (one PSUM bank of fp32)
NQ = S // SC          # 4 s-chunks

# stream_shuffle mask: swap partition pairs (2i <-> 2i+1) per quadrant
SWAP_MASK = [i + 1 if i % 2 == 0 else i - 1 for i in range(32)]

_NC_CACHE: dict = {}


def dedup_ldweights(nc) -> int:
    """Remove InstLdweights that reload the stationary already in the PE.

    The PE array keeps the stationary across MultiplyMoving passes, so a
    Ldweights whose weights AP matches the previous one on the same block
    is a no-op that still costs ~128 PE rows. Only drop instructions with
    no semaphore waits/updates; anything else on the PE queue (other than
    Matmult / event-semaphore markers) conservatively resets tracking.
    """
    from concourse import mybir

    removed = 0
    for fn in nc.m.functions:
        for blk in fn.blocks:
            insts = blk.instructions
            last_key = None
            keep = []
            i = 0
            while i < len(insts):
                inst = insts[i]
                nm = type(inst).__name__
                if nm == "InstLdweights":
                    si = inst.sync_info
                    has_update = si is not None and len(si.on_update) > 0
                    waits = list(si.on_wait) if si is not None else []
                    key = (str(inst.ins[0]), str(inst.perf_mode),
                           str(inst.is_transpose),
                           str(getattr(inst, "tile_position", None)),
                           str(getattr(inst, "tile_size", None)))
                    nxt = insts[i + 1] if i + 1 < len(insts) else None
                    if (key == last_key and not has_update
                            and (not waits
                                 or type(nxt).__name__ == "InstMatmult")):
                        if waits:
                            # push the waits down onto the paired Matmult
                            nsi = nxt.sync_info
                            if nsi is None:
                                nxt.sync_info = mybir.SyncInfo(
                                    on_wait=waits, on_update=[])
                            else:
                                nsi.on_wait = waits + list(nsi.on_wait)
                        removed += 1
                        i += 1
                        continue
                    last_key = key
                elif nm not in ("InstMatmult", "InstEventSemaphore"):
                    if getattr(inst, "engine", None) is not None and \
                            str(inst.engine) == "EngineType.PE":
                        last_key = None
                keep.append(inst)
                i += 1
            blk.instructions[:] = keep
    return removed


def build_attn_nc(iters: int = 1, phases: int = 3):
    """Build + compile the Bass module (same program for all 8 cores)."""
    import concourse.tile as tile
    from concourse import bacc, mybir

    f32 = mybir.dt.float32
    bf16 = mybir.dt.bfloat16
    AF = mybir.ActivationFunctionType
    SCALE = 1.0 / math.sqrt(DH)

    nc = bacc.Bacc("TRN2", target_bir_lowering=False, debug=False,
                   num_devices=N_CORES)

    xT = nc.dram_tensor("xT", [D, S], bf16, kind="ExternalInput").ap()
    wqT = nc.dram_tensor("wqT", [D, CW], bf16, kind="ExternalInput").ap()
    wkT = nc.dram_tensor("wkT", [D, CW], bf16, kind="ExternalInput").ap()
    wvT = nc.dram_tensor("wvT", [D, CW], bf16, kind="ExternalInput").ap()
    woT = nc.dram_tensor("woT", [CW, D], bf16, kind="ExternalInput").ap()
    bqr = nc.dram_tensor("bqr", [HPG, DH, 1], f32, kind="ExternalInput").ap()
    bkr = nc.dram_tensor("bkr", [HPG, DH, 1], f32, kind="ExternalInput").ap()
    bvb = nc.dram_tensor("bvb", [128, CW], f32, kind="ExternalInput").ap()
    cosT = nc.dram_tensor("cosT", [DH, S], bf16, kind="ExternalInput").ap()
    sinT = nc.dram_tensor("sinT", [DH, S], bf16, kind="ExternalInput").ap()
    onesd = nc.dram_tensor("onesd", [128, 128], bf16, kind="ExternalInput").ap()
    mskT = nc.dram_tensor("mskT", [4, 128, SC], bf16, kind="ExternalInput").ap()

    out = nc.dram_tensor("out", [S, D], bf16, kind="ExternalOutput").ap()

    # rearranged DRAM views for coalesced loads
    xTr = xT.rearrange("(n p) s -> p n s", p=128)          # [128, 16, 2048]
    wqr = wqT.rearrange("(n p) c -> p n c", p=128)         # [128, 16, 512]
    wkr = wkT.rearrange("(n p) c -> p n c", p=128)
    wvr = wvT.rearrange("(n p) c -> p n c", p=128)
    wor = woT.rearrange("(h p) d -> p h d", p=128)         # [128, 4, 2048]

    with tile.TileContext(nc) as tc:
        SC2 = 2 * SC
        for it in range(iters):
            with tc.tile_pool(name="const", bufs=1) as cpool, \
                 tc.tile_pool(name="qkv", bufs=1) as qkvp, \
                 tc.tile_pool(name="xqpool", bufs=1) as xqpool:
                # ---- persistent-by-tag SBUF residents ----
                # DMA issue order puts wq + x-chunk-0 first so the first
                # projection matmuls start as early as possible
                w_sb = {}
                wq_t = cpool.tile([128, NT * CW], bf16, name=f"wq_{it}",
                                  tag="wq")
                nc.sync.dma_start(wq_t[:], wqr)
                w_sb["q"] = wq_t

                # q/k rope'd chunks and v tiles, SBUF-resident through phase B
                qh = {}   # (nm, ct, C) -> [DH, SC2] bf16
                vh = {}   # (qi, st) -> [128, CW] bf16

                # ---------------- phase A: projections + RoPE ----------
                # q/k accumulate in 1024-wide PSUM tiles (2 banks each):
                # halves the matmul/Ldweights count vs 512-wide
                with tc.tile_pool(name="wkvp", bufs=1) as wkvp, \
                     tc.tile_pool(name="prawp", bufs=2) as prawp, \
                     tc.tile_pool(name="workA", bufs=1) as wkp, \
                     tc.tile_pool(name="psA", bufs=1, space="PSUM") as psA, \
                     tc.tile_pool(name="psV", bufs=4, space="PSUM") as psV:
                    # full x resident in SBUF: enables d-outer loops where
                    # one weight-slice Ldweights serves all 4 s-chunk
                    # matmuls (stationary reuse -> dedup_ldweights)
                    xf = xqpool.tile([128, NT * S], bf16,
                                     name=f"xf_{it}", tag="xf")
                    nc.sync.dma_start(xf[:], xTr)
                    for nm, dram in (("k", wkr), ("v", wvr)):
                        t = wkvp.tile([128, NT * CW], bf16,
                                      name=f"w{nm}_{it}", tag=f"w{nm}")
                        nc.sync.dma_start(t[:], dram)
                        w_sb[nm] = t
                    cos_sb = cpool.tile([DH, S], bf16, name=f"cos_{it}",
                                        tag="cos")
                    nc.sync.dma_start(cos_sb[:], cosT[:])
                    sin_sb = cpool.tile([DH, S], bf16, name=f"sin_{it}",
                                        tag="sin")
                    nc.sync.dma_start(sin_sb[:], sinT[:])
                    bq_sb, bk_sb = [], []
                    for ct in range(HPG):
                        tq = cpool.tile([DH, 1], f32, name=f"bq{ct}_{it}",
                                        tag=f"bq{ct}")
                        nc.sync.dma_start(tq[:], bqr[ct])
                        bq_sb.append(tq)
                        tk = cpool.tile([DH, 1], f32, name=f"bk{ct}_{it}",
                                        tag=f"bk{ct}")
                        nc.sync.dma_start(tk[:], bkr[ct])
                        bk_sb.append(tk)
                    bvb_sb = cpool.tile([128, CW], f32, name=f"bvb{it}",
                                        tag="bvb")
                    nc.sync.dma_start(bvb_sb[:], bvb[:])
                    ones_sb = cpool.tile([128, 128], bf16, name=f"ones{it}",
                                         tag="ones")
                    nc.sync.dma_start(ones_sb[:], onesd[:])
                    # one [128,128] triangular diag-block mask
                    # (rr-independent)
                    tri_sb = cpool.tile([128, 128], bf16, name=f"tri_{it}",
                                        tag="tri")
                    nc.sync.dma_start(tri_sb[:], mskT[0, :, 0:128])

                    def qk_ct(nm, ct, bias_t):
                        # d-outer: one stationary w-slice per d feeds the 4
                        # s-chunk matmuls; all 4 chunks accumulate in one
                        # 4-bank PSUM tile (each matmul stays <=512 wide)
                        ps = psA.tile([128, NQ * SC], f32,
                                      name=f"ps{nm}{ct}_{it}", tag="ps")
                        for d in range(NT):
                            wsl = w_sb[nm][:, d * CW + ct * DH:
                                           d * CW + (ct + 1) * DH]
                            for qi in range(NQ):
                                nc.tensor.matmul(
                                    ps[:, qi * SC:(qi + 1) * SC], wsl,
                                    xf[:, d * S + qi * SC:
                                       d * S + (qi + 1) * SC],
                                    start=(d == 0), stop=(d == NT - 1),
                                    skip_group_check=True)
                        praw = prawp.tile(
                            [128, NQ * SC], bf16,
                            name=f"praw{nm}{ct}_{it}", tag="praw")
                        nc.vector.tensor_scalar_add(
                            praw[:], ps[:], bias_t[:])
                        return praw

                    def rope_ct(nm, ct, praw):
                        # RoPE on DVE only, whole-row [128, 2048] ops:
                        # rotate-half is a pairwise partition swap
                        # (stream_shuffle, quadrant-local) with the sign
                        # folded into sinT host-side
                        pro = qkvp.tile([DH, NQ * SC], bf16,
                                        name=f"pro{nm}{ct}_{it}",
                                        tag=f"pro{nm}{ct}")
                        sh = wkp.tile([128, NQ * SC], bf16,
                                      name=f"sh{nm}{ct}_{it}", tag="sh")
                        nc.vector.stream_shuffle(sh[:], praw[:], SWAP_MASK)
                        m1 = wkp.tile([128, NQ * SC], bf16,
                                      name=f"m1{nm}{ct}_{it}", tag="m1")
                        nc.vector.tensor_mul(m1[:], praw[:], cos_sb[:])
                        m2 = wkp.tile([128, NQ * SC], bf16,
                                      name=f"m2{nm}{ct}_{it}", tag="m2")
                        nc.vector.tensor_mul(m2[:], sh[:], sin_sb[:])
                        nc.vector.tensor_add(pro[:], m1[:], m2[:])
                        for qi in range(NQ):
                            qh[(nm, ct, qi)] = (pro, qi * SC)

                    def v_pair(sp):
                        psa = psV.tile([128, CW], f32,
                                       name=f"psv{sp}_{it}", tag="psv")
                        psb = psV.tile([128, CW], f32,
                                       name=f"psv{sp+1}_{it}", tag="psv")
                        for d in range(NT):
                            xc = d * S
                            nc.tensor.matmul(
                                psa[:],
                                xf[:, xc + sp * 128:xc + (sp + 1) * 128],
                                w_sb["v"][:, d * CW:(d + 1) * CW],
                                start=(d == 0), stop=(d == NT - 1))
                            nc.tensor.matmul(
                                psb[:],
                                xf[:, xc + (sp + 1) * 128:
                                   xc + (sp + 2) * 128],
                                w_sb["v"][:, d * CW:(d + 1) * CW],
                                start=(d == 0), stop=(d == NT - 1))
                        for st, ps in ((sp, psa), (sp + 1, psb)):
                            vt = qkvp.tile([128, CW], bf16,
                                           name=f"vt{st}_{it}",
                                           tag=f"vt{st}")
                            nc.vector.tensor_add(vt[:], ps[:], bvb_sb[:])
                            vh[(st // 4, st % 4)] = vt

                    # interleave v pair-groups (own PSUM banks) between the
                    # 8 q/k ct-groups so each group's PSUM drain is hidden
                    # behind the next group's matmuls
                    sp = 0
                    for nm, biases in (("q", bq_sb), ("k", bk_sb)):
                        for ct in range(HPG):
                            praw = qk_ct(nm, ct, biases[ct])
                            v_pair(sp)
                            sp += 2
                            rope_ct(nm, ct, praw)

                # ---------------- phase B: attention -------------------
                if phases < 2:
                    # debug mode: phase A only
                    continue
                with tc.tile_pool(name="aopool", bufs=1) as aopool, \
                     tc.tile_pool(name="wop", bufs=1) as wop:
                    aoT = aopool.tile([128, HPG * S], bf16, name=f"aoT_{it}",
                                      tag="aoT")
                    wo_sb = wop.tile([128, HPG * D], bf16, name=f"wo_{it}",
                                     tag="wo")
                    nc.sync.dma_start(wo_sb[:], wor)
                    with tc.tile_pool(name="atpool", bufs=8) as atpool, \
                         tc.tile_pool(name="recpool", bufs=2) as recpool, \
                         tc.tile_pool(name="psO", bufs=2, space="PSUM") as psO, \
                         tc.tile_pool(name="psS", bufs=4, space="PSUM") as psS:
                        for h in range(HPG):
                            qh_c = [qh[("q", h, qi)] for qi in range(NQ)]
                            kh_c = [qh[("k", h, qi)] for qi in range(NQ)]
                            vh_t = [vh[(t_ // 4, t_ % 4)]
                                    for t_ in range(NT)]
                            for c in range(NQ):
                                q0 = c * SC
                                ntile = 4 * c + 4
                                oT = psO.tile([DH, SC], f32,
                                              name=f"oT{h}{c}_{it}", tag="oT")
                                dn = psO.tile([128, SC], f32,
                                              name=f"dn{h}{c}_{it}", tag="dn")
                                for t_ in range(ntile):
                                    rr = t_ - 4 * c
                                    n0 = rr * 128 if rr > 0 else 0
                                    sps = psS.tile(
                                        [128, SC], f32,
                                        name=f"sps{h}{c}{t_}_{it}", tag="sps")
                                    kt, ko = kh_c[t_ // 4]
                                    qt, qo = qh_c[c]
                                    nc.tensor.matmul(
                                        sps[:, n0:],
                                        kt[:, ko + (t_ % 4) * 128:
                                           ko + (t_ % 4 + 1) * 128],
                                        qt[:, qo + n0:qo + SC],
                                        start=True, stop=True)
                                    at = atpool.tile(
                                        [128, SC], bf16,
                                        name=f"at{h}{c}{t_}_{it}", tag="at")
                                    nc.scalar.activation(
                                        at[:, n0:], sps[:, n0:],
                                        AF.Exp, bias=0.0, scale=SCALE)
                                    if rr >= 0:
                                        # mask the triangular diagonal block
                                        # in place (cols [0,n0) already
                                        # skipped entirely); the 128x128
                                        # triangle is rr-independent
                                        nc.vector.tensor_mul(
                                            at[:, n0:n0 + 128],
                                            at[:, n0:n0 + 128],
                                            tri_sb[:])
                                    nc.tensor.matmul(
                                        oT[:, n0:],
                                        vh_t[t_][:, h * DH:(h + 1) * DH],
                                        at[:, n0:],
                                        start=(t_ == 0),
                                        stop=(t_ == ntile - 1),
                                        skip_group_check=True)
                                    nc.tensor.matmul(
                                        dn[:, n0:], ones_sb[:], at[:, n0:],
                                        start=(t_ == 0),
                                        stop=(t_ == ntile - 1),
                                        skip_group_check=True)
                                rec = recpool.tile([128, SC], f32,
                                                   name=f"rec{h}{c}_{it}",
                                                   tag="rec")
                                nc.vector.reciprocal(rec[:], dn[:])
                                nc.vector.tensor_mul(
                                    aoT[:, h * S + q0:h * S + q0 + SC],
                                    oT[:], rec[:])

                    # ------------ phase C: output projection ------------
                    if phases < 3:
                        continue
                    with tc.tile_pool(name="outpool", bufs=2) as outpool, \
                         tc.tile_pool(name="psC", bufs=8, space="PSUM") as psC:
                        for st in range(NT):
                            ops = []
                            for dc in range(4):
                                op = psC.tile([128, SC], f32,
                                              name=f"op{st}{dc}_{it}",
                                              tag="op")
                                ops.append(op)
                            for hh in range(HPG):
                                lhs = aoT[:, hh * S + st * 128:
                                          hh * S + (st + 1) * 128]
                                for dc in range(4):
                                    nc.tensor.matmul(
                                        ops[dc][:], lhs,
                                        wo_sb[:, hh * D + dc * SC:
                                              hh * D + (dc + 1) * SC],
                                        start=(hh == 0), stop=(hh == HPG - 1))
                            ot = outpool.tile([128, D], bf16,
                                              name=f"ot{st}_{it}", tag="ot")
                            for dc in range(4):
                                # split PSUM->SBUF copies across DVE and ACT
                                dst = ot[:, dc * SC:(dc + 1) * SC]
                                if dc % 2 == 0:
                                    nc.vector.tensor_copy(dst, ops[dc][:])
                                else:
                                    nc.scalar.copy(dst, ops[dc][:])
                            # store on the ACT queue (idle in phase C) so
                            # next iteration's loads on SP aren't queued
                            # behind these stores (cross-iter PE stall)
                            nc.scalar.dma_start(
                                out[st * 128:(st + 1) * 128, :], ot[:])
    nc.compile()
    dedup_ldweights(nc)
    return nc


def host_prep(inputs: dict) -> list:
    """Build per-core input maps (host-side sharding + relayout)."""
    import ml_dtypes
    bf16 = ml_dtypes.bfloat16

    x = np.asarray(inputs["x"], dtype=np.float32)
    wq = np.asarray(inputs["wq"], dtype=np.float32)
    wk = np.asarray(inputs["wk"], dtype=np.float32)
    wv = np.asarray(inputs["wv"], dtype=np.float32)
    wo = np.asarray(inputs["wo"], dtype=np.float32)
    bq = np.asarray(inputs["bq"], dtype=np.float32)
    bk = np.asarray(inputs["bk"], dtype=np.float32)
    bv = np.asarray(inputs["bv"], dtype=np.float32)
    mask = np.asarray(inputs["mask"])

    inv = 1.0 / (10000.0 ** (np.arange(0, DH, 2, dtype=np.float64) / DH))
    ang = np.arange(S, dtype=np.float64)[:, None] * inv[None, :]
    sin = np.repeat(np.sin(ang), 2, axis=1).astype(np.float32)
    cos = np.repeat(np.cos(ang), 2, axis=1).astype(np.float32)
    cosT = np.ascontiguousarray(cos.T).astype(bf16)
    # rotate-half is done as a pure pair-swap (stream_shuffle); fold the
    # sign of -x[2i+1] into sin: even dh rows get -sin
    sinS = np.ascontiguousarray(sin.T)
    sinS[0::2, :] *= -1.0
    sinT = sinS.astype(bf16)

    m2 = mask[0, 0]
    mskT = np.zeros((4, 128, SC), np.float32)
    for rr in range(4):
        # keep[i, j] = not masked(q=j, k=rr*128+i)
        mskT[rr] = (~m2[:SC, rr * 128:(rr + 1) * 128]).T.astype(np.float32)
    mskT = mskT.astype(bf16)

    xTb = [np.ascontiguousarray(x[b].T).astype(bf16) for b in range(B)]
    in_maps = []
    for core in range(N_CORES):
        b, g = divmod(core, G)
        c0 = g * CW
        in_maps.append({
            "xT": xTb[b],
            "wqT": np.ascontiguousarray(wq[c0:c0 + CW, :].T).astype(bf16),
            "wkT": np.ascontiguousarray(wk[c0:c0 + CW, :].T).astype(bf16),
            "wvT": np.ascontiguousarray(wv[c0:c0 + CW, :].T).astype(bf16),
            "woT": np.ascontiguousarray(wo[:, c0:c0 + CW].T).astype(bf16),
            "bqr": np.ascontiguousarray(
                bq[c0:c0 + CW].reshape(HPG, DH, 1)),
            "bkr": np.ascontiguousarray(
                bk[c0:c0 + CW].reshape(HPG, DH, 1)),
            "bvb": np.ascontiguousarray(
                np.broadcast_to(bv[c0:c0 + CW], (128, CW))),
            "cosT": cosT,
            "sinT": sinT,
            "onesd": np.ones((128, 128), np.float32).astype(bf16),
            "mskT": mskT,
        })
    return in_maps


def _get_nc():
    if "nc" not in _NC_CACHE:
        _NC_CACHE["nc"] = build_attn_nc(iters=1)
    return _NC_CACHE["nc"]


def kernel(**inputs) -> np.ndarray:
    from concourse.bass_utils import run_bass_kernel_spmd

    nc = _get_nc()
    in_maps = host_prep(inputs)
    res = run_bass_kernel_spmd(nc, in_maps, core_ids=list(range(N_CORES)))
    bo = np.asarray(inputs["bo"], dtype=np.float32)
    outp = np.zeros((B, S, D), np.float32)
    for core in range(N_CORES):
        outp[core // G] += np.asarray(res.results[core]["out"],
                                      dtype=np.float32)
    outp += bo[None, None, :]
    return outp


# revision 32
# speedup vs baseline: 1.1672x; 1.0515x over previous
"""Trainium2 Bass kernel for 16-head causal self-attention with RoPE.

Problem: x:[2,2048,2048] -> MHA(wq,wk,wv,wo, causal mask, RoPE) -> [2,2048,2048].

Sharding (8 NeuronCores): core = b*4 + g, where b in {0,1} is the batch
(data parallel) and g in {0..3} is a head group of 4 heads (tensor parallel
over the 16 heads / 2048 channels: group g owns channels [g*512, (g+1)*512)).

Design (HW ~363us vs ~509us f32r baseline, same harness; sim ~314us):
  - bf16 on every DMA/SBUF path (fp32 PSUM accumulation): halves HBM bytes,
    removes the f32r narrow-matmul 4x penalty, enables DVE 2x/4x modes.
  - q/k/v stay SBUF-resident between projection and attention (no DRAM
    roundtrip): saves 24MB/core of DMA traffic.
  - coalesced DMAs (~34 per iteration vs ~350): HWDGE charges ~650ns of
    serialized descriptor-gen per DMA instruction.
  - full x resident in SBUF; projections run d-outer so one Ldweights
    (stationary load) serves the 4 s-chunk matmuls, and dedup_ldweights()
    strips the redundant reloads post-compile (PE keeps the stationary) —
    on HW each Ldweights+Matmult pair costs well above the 1-cycle/row
    model, so instruction count matters as much as MACs.
  - RoPE runs entirely on DVE: rotate-half = stream_shuffle partition-pair
    swap with the sign folded into sin host-side (no rotation matmul, no
    PSUM round-trip), applied on whole [128,2048] rows.
  - output stores go on the ACT queue and xf/wq SBUF tiles persist across
    the iteration so next iteration's loads are neither queued behind this
    iteration's stores nor blocked by SBUF-address reuse (those two effects
    serialized iterations by ~30us each).
  - phase C PSUM->SBUF copies split DVE/ACT so neither engine serializes
    the output stream.
Per-core phases:
  A: q/k = RoPE(x @ wq_loc.T), v = x @ wv_loc.T   (bf16 matmuls, SBUF out)
  B: per head, scoresT = kT.T-tiles @ qT, exp on ACT (scale folded),
     causal strictly-upper 128x512 tiles skipped, diagonal masked by
     in-place multiply with one shared 128x128 triangle, PV + ones-matmul
     denominator accumulate in PSUM, one reciprocal+mul per (head, chunk).
  C: partial out = ao @ wo_loc.T -> DRAM (bf16)
Host: out[b] = sum of the 4 group partials + bo.
Matmul moving dim is capped at 512 (one PSUM bank) by the ISA — wider
PSUM accumulators are written as multiple <=512-wide matmuls.
"""

import math
import sys

sys.path.insert(0, "/opt/trn_rl_repo")

import numpy as np

N_CORES = 8
B, S, D = 2, 2048, 2048
H, DH = 16, 128
G = 4                 # head groups (tensor-parallel factor per batch)
HPG = H // G          # heads per group = 4
CW = HPG * DH         # channels per group = 512
NT = S // 128         # 16 s-tiles
SC = 512              # free-dim chunk (one PSUM bank of fp32)
NQ = S // SC          # 4 s-chunks

# stream_shuffle mask: swap partition pairs (2i <-> 2i+1) per quadrant
SWAP_MASK = [i + 1 if i % 2 == 0 else i - 1 for i in range(32)]

_NC_CACHE: dict = {}


def dedup_ldweights(nc) -> int:
    """Remove InstLdweights that reload the stationary already in the PE.

    The PE array keeps the stationary across MultiplyMoving passes, so a
    Ldweights whose weights AP matches the previous one on the same block
    is a no-op that still costs ~128 PE rows. Only drop instructions with
    no semaphore waits/updates; anything else on the PE queue (other than
    Matmult / event-semaphore markers) conservatively resets tracking.
    """
    from concourse import mybir

    removed = 0
    for fn in nc.m.functions:
        for blk in fn.blocks:
            insts = blk.instructions
            last_key = None
            keep = []
            i = 0
            while i < len(insts):
                inst = insts[i]
                nm = type(inst).__name__
                if nm == "InstLdweights":
                    si = inst.sync_info
                    has_update = si is not None and len(si.on_update) > 0
                    waits = list(si.on_wait) if si is not None else []
                    key = (str(inst.ins[0]), str(inst.perf_mode),
                           str(inst.is_transpose),
                           str(getattr(inst, "tile_position", None)),
                           str(getattr(inst, "tile_size", None)))
                    nxt = insts[i + 1] if i + 1 < len(insts) else None
                    if (key == last_key and not has_update
                            and (not waits
                                 or type(nxt).__name__ == "InstMatmult")):
                        if waits:
                            # push the waits down onto the paired Matmult
                            nsi = nxt.sync_info
                            if nsi is None:
                                nxt.sync_info = mybir.SyncInfo(
                                    on_wait=waits, on_update=[])
                            else:
                                nsi.on_wait = waits + list(nsi.on_wait)
                        removed += 1
                        i += 1
                        continue
                    last_key = key
                elif nm not in ("InstMatmult", "InstEventSemaphore"):
                    if getattr(inst, "engine", None) is not None and \
                            str(inst.engine) == "EngineType.PE":
                        last_key = None
                keep.append(inst)
                i += 1
            blk.instructions[:] = keep
    return removed


def build_attn_nc(iters: int = 1, phases: int = 3):
    """Build + compile the Bass module (same program for all 8 cores)."""
    import concourse.tile as tile
    from concourse import bacc, mybir

    f32 = mybir.dt.float32
    bf16 = mybir.dt.bfloat16
    AF = mybir.ActivationFunctionType
    SCALE = 1.0 / math.sqrt(DH)

    nc = bacc.Bacc("TRN2", target_bir_lowering=False, debug=False,
                   num_devices=N_CORES)

    xT = nc.dram_tensor("xT", [D, S], bf16, kind="ExternalInput").ap()
    wqT = nc.dram_tensor("wqT", [D, CW], bf16, kind="ExternalInput").ap()
    wkT = nc.dram_tensor("wkT", [D, CW], bf16, kind="ExternalInput").ap()
    wvT = nc.dram_tensor("wvT", [D, CW], bf16, kind="ExternalInput").ap()
    woT = nc.dram_tensor("woT", [CW, D], bf16, kind="ExternalInput").ap()
    bqr = nc.dram_tensor("bqr", [HPG, DH, 1], f32, kind="ExternalInput").ap()
    bkr = nc.dram_tensor("bkr", [HPG, DH, 1], f32, kind="ExternalInput").ap()
    bvb = nc.dram_tensor("bvb", [128, CW], f32, kind="ExternalInput").ap()
    cosT = nc.dram_tensor("cosT", [DH, S], bf16, kind="ExternalInput").ap()
    sinT = nc.dram_tensor("sinT", [DH, S], bf16, kind="ExternalInput").ap()
    onesd = nc.dram_tensor("onesd", [128, 128], bf16, kind="ExternalInput").ap()
    mskT = nc.dram_tensor("mskT", [4, 128, SC], bf16, kind="ExternalInput").ap()

    out = nc.dram_tensor("out", [S, D], bf16, kind="ExternalOutput").ap()

    # rearranged DRAM views for coalesced loads
    xTr = xT.rearrange("(n p) s -> p n s", p=128)          # [128, 16, 2048]
    wqr = wqT.rearrange("(n p) c -> p n c", p=128)         # [128, 16, 512]
    wkr = wkT.rearrange("(n p) c -> p n c", p=128)
    wvr = wvT.rearrange("(n p) c -> p n c", p=128)
    wor = woT.rearrange("(h p) d -> p h d", p=128)         # [128, 4, 2048]

    with tile.TileContext(nc) as tc:
        SC2 = 2 * SC
        for it in range(iters):
            with tc.tile_pool(name="const", bufs=1) as cpool, \
                 tc.tile_pool(name="qkv", bufs=1) as qkvp, \
                 tc.tile_pool(name="xqpool", bufs=1) as xqpool:
                # ---- persistent-by-tag SBUF residents ----
                # DMA issue order puts wq + x-chunk-0 first so the first
                # projection matmuls start as early as possible
                w_sb = {}
                wq_t = cpool.tile([128, NT * CW], bf16, name=f"wq_{it}",
                                  tag="wq")
                nc.sync.dma_start(wq_t[:], wqr)
                w_sb["q"] = wq_t

                # q/k rope'd chunks and v tiles, SBUF-resident through phase B
                qh = {}   # (nm, ct, C) -> [DH, SC2] bf16
                vh = {}   # (qi, st) -> [128, CW] bf16

                # ---------------- phase A: projections + RoPE ----------
                # q/k accumulate in 1024-wide PSUM tiles (2 banks each):
                # halves the matmul/Ldweights count vs 512-wide
                with tc.tile_pool(name="wkvp", bufs=1) as wkvp, \
                     tc.tile_pool(name="prawp", bufs=2) as prawp, \
                     tc.tile_pool(name="workA", bufs=1) as wkp, \
                     tc.tile_pool(name="psA", bufs=1, space="PSUM") as psA, \
                     tc.tile_pool(name="psV", bufs=4, space="PSUM") as psV:
                    # full x resident in SBUF: enables d-outer loops where
                    # one weight-slice Ldweights serves all 4 s-chunk
                    # matmuls (stationary reuse -> dedup_ldweights)
                    xf = xqpool.tile([128, NT * S], bf16,
                                     name=f"xf_{it}", tag="xf")
                    nc.sync.dma_start(xf[:], xTr)
                    for nm, dram in (("k", wkr), ("v", wvr)):
                        t = wkvp.tile([128, NT * CW], bf16,
                                      name=f"w{nm}_{it}", tag=f"w{nm}")
                        nc.sync.dma_start(t[:], dram)
                        w_sb[nm] = t
                    cos_sb = cpool.tile([DH, S], bf16, name=f"cos_{it}",
                                        tag="cos")
                    nc.sync.dma_start(cos_sb[:], cosT[:])
                    sin_sb = cpool.tile([DH, S], bf16, name=f"sin_{it}",
                                        tag="sin")
                    nc.sync.dma_start(sin_sb[:], sinT[:])
                    bq_sb, bk_sb = [], []
                    for ct in range(HPG):
                        tq = cpool.tile([DH, 1], f32, name=f"bq{ct}_{it}",
                                        tag=f"bq{ct}")
                        nc.sync.dma_start(tq[:], bqr[ct])
                        bq_sb.append(tq)
                        tk = cpool.tile([DH, 1], f32, name=f"bk{ct}_{it}",
                                        tag=f"bk{ct}")
                        nc.sync.dma_start(tk[:], bkr[ct])
                        bk_sb.append(tk)
                    bvb_sb = cpool.tile([128, CW], f32, name=f"bvb{it}",
                                        tag="bvb")
                    nc.sync.dma_start(bvb_sb[:], bvb[:])
                    ones_sb = cpool.tile([128, 128], bf16, name=f"ones{it}",
                                         tag="ones")
                    nc.sync.dma_start(ones_sb[:], onesd[:])
                    # one [128,128] triangular diag-block mask
                    # (rr-independent)
                    tri_sb = cpool.tile([128, 128], bf16, name=f"tri_{it}",
                                        tag="tri")
                    nc.sync.dma_start(tri_sb[:], mskT[0, :, 0:128])

                    def qk_ct(nm, ct, bias_t):
                        # d-outer: one stationary w-slice per d feeds the 4
                        # s-chunk matmuls; all 4 chunks accumulate in one
                        # 4-bank PSUM tile (each matmul stays <=512 wide)
                        ps = psA.tile([128, NQ * SC], f32,
                                      name=f"ps{nm}{ct}_{it}", tag="ps")
                        for d in range(NT):
                            wsl = w_sb[nm][:, d * CW + ct * DH:
                                           d * CW + (ct + 1) * DH]
                            for qi in range(NQ):
                                nc.tensor.matmul(
                                    ps[:, qi * SC:(qi + 1) * SC], wsl,
                                    xf[:, d * S + qi * SC:
                                       d * S + (qi + 1) * SC],
                                    start=(d == 0), stop=(d == NT - 1),
                                    skip_group_check=True)
                        praw = prawp.tile(
                            [128, NQ * SC], bf16,
                            name=f"praw{nm}{ct}_{it}", tag="praw")
                        # drain on ACT (idle during phase A; same per-
                        # partition bias semantics) so DVE stays free for
                        # the RoPE chain and the PSUM WAR clears sooner
                        nc.scalar.add(praw[:], ps[:], bias_t[:])
                        return praw

                    def rope_ct(nm, ct, praw):
                        # RoPE on DVE only, whole-row [128, 2048] ops:
                        # rotate-half is a pairwise partition swap
                        # (stream_shuffle, quadrant-local) with the sign
                        # folded into sinT host-side
                        pro = qkvp.tile([DH, NQ * SC], bf16,
                                        name=f"pro{nm}{ct}_{it}",
                                        tag=f"pro{nm}{ct}")
                        sh = wkp.tile([128, NQ * SC], bf16,
                                      name=f"sh{nm}{ct}_{it}", tag="sh")
                        nc.vector.stream_shuffle(sh[:], praw[:], SWAP_MASK)
                        m1 = wkp.tile([128, NQ * SC], bf16,
                                      name=f"m1{nm}{ct}_{it}", tag="m1")
                        nc.vector.tensor_mul(m1[:], praw[:], cos_sb[:])
                        m2 = wkp.tile([128, NQ * SC], bf16,
                                      name=f"m2{nm}{ct}_{it}", tag="m2")
                        nc.vector.tensor_mul(m2[:], sh[:], sin_sb[:])
                        nc.vector.tensor_add(pro[:], m1[:], m2[:])
                        for qi in range(NQ):
                            qh[(nm, ct, qi)] = (pro, qi * SC)

                    def v_pair(sp):
                        psa = psV.tile([128, CW], f32,
                                       name=f"psv{sp}_{it}", tag="psv")
                        psb = psV.tile([128, CW], f32,
                                       name=f"psv{sp+1}_{it}", tag="psv")
                        for d in range(NT):
                            xc = d * S
                            nc.tensor.matmul(
                                psa[:],
                                xf[:, xc + sp * 128:xc + (sp + 1) * 128],
                                w_sb["v"][:, d * CW:(d + 1) * CW],
                                start=(d == 0), stop=(d == NT - 1))
                            nc.tensor.matmul(
                                psb[:],
                                xf[:, xc + (sp + 1) * 128:
                                   xc + (sp + 2) * 128],
                                w_sb["v"][:, d * CW:(d + 1) * CW],
                                start=(d == 0), stop=(d == NT - 1))
                        for st, ps in ((sp, psa), (sp + 1, psb)):
                            vt = qkvp.tile([128, CW], bf16,
                                           name=f"vt{st}_{it}",
                                           tag=f"vt{st}")
                            nc.vector.tensor_add(vt[:], ps[:], bvb_sb[:])
                            vh[(st // 4, st % 4)] = vt

                    # interleave v pair-groups (own PSUM banks) between the
                    # 8 q/k ct-groups so each group's PSUM drain is hidden
                    # behind the next group's matmuls
                    sp = 0
                    for nm, biases in (("q", bq_sb), ("k", bk_sb)):
                        for ct in range(HPG):
                            praw = qk_ct(nm, ct, biases[ct])
                            v_pair(sp)
                            sp += 2
                            rope_ct(nm, ct, praw)

                # ---------------- phase B: attention -------------------
                if phases < 2:
                    # debug mode: phase A only
                    continue
                with tc.tile_pool(name="aopool", bufs=1) as aopool, \
                     tc.tile_pool(name="wop", bufs=1) as wop:
                    aoT = aopool.tile([128, HPG * S], bf16, name=f"aoT_{it}",
                                      tag="aoT")
                    wo_sb = wop.tile([128, HPG * D], bf16, name=f"wo_{it}",
                                     tag="wo")
                    nc.sync.dma_start(wo_sb[:], wor)
                    with tc.tile_pool(name="atpool", bufs=8) as atpool, \
                         tc.tile_pool(name="recpool", bufs=2) as recpool, \
                         tc.tile_pool(name="psO", bufs=2, space="PSUM") as psO, \
                         tc.tile_pool(name="psS", bufs=4, space="PSUM") as psS:
                        for h in range(HPG):
                            qh_c = [qh[("q", h, qi)] for qi in range(NQ)]
                            kh_c = [qh[("k", h, qi)] for qi in range(NQ)]
                            vh_t = [vh[(t_ // 4, t_ % 4)]
                                    for t_ in range(NT)]
                            for c in range(NQ):
                                q0 = c * SC
                                ntile = 4 * c + 4
                                oT = psO.tile([DH, SC], f32,
                                              name=f"oT{h}{c}_{it}", tag="oT")
                                dn = psO.tile([128, SC], f32,
                                              name=f"dn{h}{c}_{it}", tag="dn")
                                for t_ in range(ntile):
                                    rr = t_ - 4 * c
                                    n0 = rr * 128 if rr > 0 else 0
                                    sps = psS.tile(
                                        [128, SC], f32,
                                        name=f"sps{h}{c}{t_}_{it}", tag="sps")
                                    kt, ko = kh_c[t_ // 4]
                                    qt, qo = qh_c[c]
                                    nc.tensor.matmul(
                                        sps[:, n0:],
                                        kt[:, ko + (t_ % 4) * 128:
                                           ko + (t_ % 4 + 1) * 128],
                                        qt[:, qo + n0:qo + SC],
                                        start=True, stop=True)
                                    at = atpool.tile(
                                        [128, SC], bf16,
                                        name=f"at{h}{c}{t_}_{it}", tag="at")
                                    nc.scalar.activation(
                                        at[:, n0:], sps[:, n0:],
                                        AF.Exp, bias=0.0, scale=SCALE)
                                    if rr >= 0:
                                        # mask the triangular diagonal block
                                        # in place (cols [0,n0) already
                                        # skipped entirely); the 128x128
                                        # triangle is rr-independent
                                        nc.vector.tensor_mul(
                                            at[:, n0:n0 + 128],
                                            at[:, n0:n0 + 128],
                                            tri_sb[:])
                                    nc.tensor.matmul(
                                        oT[:, n0:],
                                        vh_t[t_][:, h * DH:(h + 1) * DH],
                                        at[:, n0:],
                                        start=(t_ == 0),
                                        stop=(t_ == ntile - 1),
                                        skip_group_check=True)
                                    nc.tensor.matmul(
                                        dn[:, n0:], ones_sb[:], at[:, n0:],
                                        start=(t_ == 0),
                                        stop=(t_ == ntile - 1),
                                        skip_group_check=True)
                                rec = recpool.tile([128, SC], f32,
                                                   name=f"rec{h}{c}_{it}",
                                                   tag="rec")
                                nc.vector.reciprocal(rec[:], dn[:])
                                nc.vector.tensor_mul(
                                    aoT[:, h * S + q0:h * S + q0 + SC],
                                    oT[:], rec[:])

                    # ------------ phase C: output projection ------------
                    if phases < 3:
                        continue
                    with tc.tile_pool(name="outpool", bufs=2) as outpool, \
                         tc.tile_pool(name="psC", bufs=8, space="PSUM") as psC:
                        for st in range(NT):
                            ops = []
                            for dc in range(4):
                                op = psC.tile([128, SC], f32,
                                              name=f"op{st}{dc}_{it}",
                                              tag="op")
                                ops.append(op)
                            for hh in range(HPG):
                                lhs = aoT[:, hh * S + st * 128:
                                          hh * S + (st + 1) * 128]
                                for dc in range(4):
                                    nc.tensor.matmul(
                                        ops[dc][:], lhs,
                                        wo_sb[:, hh * D + dc * SC:
                                              hh * D + (dc + 1) * SC],
                                        start=(hh == 0), stop=(hh == HPG - 1))
                            ot = outpool.tile([128, D], bf16,
                                              name=f"ot{st}_{it}", tag="ot")
                            for dc in range(4):
                                # split PSUM->SBUF copies across DVE and ACT
                                dst = ot[:, dc * SC:(dc + 1) * SC]
                                if dc % 2 == 0:
                                    nc.vector.tensor_copy(dst, ops[dc][:])
                                else:
                                    nc.scalar.copy(dst, ops[dc][:])
                            # store on the ACT queue (idle in phase C) so
                            # next iteration's loads on SP aren't queued
                            # behind these stores (cross-iter PE stall)
                            nc.scalar.dma_start(
                                out[st * 128:(st + 1) * 128, :], ot[:])
    nc.compile()
    dedup_ldweights(nc)
    return nc


def host_prep(inputs: dict) -> list:
    """Build per-core input maps (host-side sharding + relayout)."""
    import ml_dtypes
    bf16 = ml_dtypes.bfloat16

    x = np.asarray(inputs["x"], dtype=np.float32)
    wq = np.asarray(inputs["wq"], dtype=np.float32)
    wk = np.asarray(inputs["wk"], dtype=np.float32)
    wv = np.asarray(inputs["wv"], dtype=np.float32)
    wo = np.asarray(inputs["wo"], dtype=np.float32)
    bq = np.asarray(inputs["bq"], dtype=np.float32)
    bk = np.asarray(inputs["bk"], dtype=np.float32)
    bv = np.asarray(inputs["bv"], dtype=np.float32)
    mask = np.asarray(inputs["mask"])

    inv = 1.0 / (10000.0 ** (np.arange(0, DH, 2, dtype=np.float64) / DH))
    ang = np.arange(S, dtype=np.float64)[:, None] * inv[None, :]
    sin = np.repeat(np.sin(ang), 2, axis=1).astype(np.float32)
    cos = np.repeat(np.cos(ang), 2, axis=1).astype(np.float32)
    cosT = np.ascontiguousarray(cos.T).astype(bf16)
    # rotate-half is done as a pure pair-swap (stream_shuffle); fold the
    # sign of -x[2i+1] into sin: even dh rows get -sin
    sinS = np.ascontiguousarray(sin.T)
    sinS[0::2, :] *= -1.0
    sinT = sinS.astype(bf16)

    m2 = mask[0, 0]
    mskT = np.zeros((4, 128, SC), np.float32)
    for rr in range(4):
        # keep[i, j] = not masked(q=j, k=rr*128+i)
        mskT[rr] = (~m2[:SC, rr * 128:(rr + 1) * 128]).T.astype(np.float32)
    mskT = mskT.astype(bf16)

    xTb = [np.ascontiguousarray(x[b].T).astype(bf16) for b in range(B)]
    in_maps = []
    for core in range(N_CORES):
        b, g = divmod(core, G)
        c0 = g * CW
        in_maps.append({
            "xT": xTb[b],
            "wqT": np.ascontiguousarray(wq[c0:c0 + CW, :].T).astype(bf16),
            "wkT": np.ascontiguousarray(wk[c0:c0 + CW, :].T).astype(bf16),
            "wvT": np.ascontiguousarray(wv[c0:c0 + CW, :].T).astype(bf16),
            "woT": np.ascontiguousarray(wo[:, c0:c0 + CW].T).astype(bf16),
            "bqr": np.ascontiguousarray(
                bq[c0:c0 + CW].reshape(HPG, DH, 1)),
            "bkr": np.ascontiguousarray(
                bk[c0:c0 + CW].reshape(HPG, DH, 1)),
            "bvb": np.ascontiguousarray(
                np.broadcast_to(bv[c0:c0 + CW], (128, CW))),
            "cosT": cosT,
            "sinT": sinT,
            "onesd": np.ones((128, 128), np.float32).astype(bf16),
            "mskT": mskT,
        })
    return in_maps


def _get_nc():
    if "nc" not in _NC_CACHE:
        _NC_CACHE["nc"] = build_attn_nc(iters=1)
    return _NC_CACHE["nc"]


def kernel(**inputs) -> np.ndarray:
    from concourse.bass_utils import run_bass_kernel_spmd

    nc = _get_nc()
    in_maps = host_prep(inputs)
    res = run_bass_kernel_spmd(nc, in_maps, core_ids=list(range(N_CORES)))
    bo = np.asarray(inputs["bo"], dtype=np.float32)
    outp = np.zeros((B, S, D), np.float32)
    for core in range(N_CORES):
        outp[core // G] += np.asarray(res.results[core]["out"],
                                      dtype=np.float32)
    outp += bo[None, None, :]
    return outp
